# revision 19
# baseline (speedup 1.0000x reference)
"""Trainium2 Bass kernel for word2vec-style binary log loss (negative sampling).

loss = sum_n -logsig(h_n . E[pos_n]) + sum_n mean_k -logsig(-h_n . E[neg_nk])
     = sum over all (n,pair) of w * softplus(sigma * score)
       with (sigma, w) = (-1, 1) for the positive pair, (+1, 1/20) for negatives.

Strategy: vocab-parallel. The [V, D] table (cast to bf16 on host) is sharded
row-wise across the 8 cores (125k rows / 32 MB each); each core processes the
~43k (sample, table-row) pairs whose row index lands in its shard.  Pairs are
sorted by local row index and grouped into 16384-row chunks so indices fit
dma_gather's int16 contract; each chunk's table rows are fetched with an
InstDMAGatherAnt call (the scattered-read half of the problem stays on
device).  The hidden state is tiny (16384 x 128), so the host pre-replicates
hidden rows into pair-slot order and the kernel streams them with plain
contiguous HWDGE DMAs - this halves the per-packet DMA cost that dominates
the gather-bound roofline.  Gathered rows land as [pair%128 -> partition,
pair//128 -> block]; scores are an elementwise bf16 multiply + one reduce
over d.  Softplus runs on ScalarE via relu(x) + log1p(exp(-|x|)).  Each core
emits a [128,1] partial sum; the host adds the 8*128 partials.
"""

import os
import sys

for _p in ("/opt/trn_rl_repo", "/root/.axon_site/_ro/trn_rl_repo"):
    if os.path.isdir(_p) and _p not in sys.path:
        sys.path.insert(0, _p)

import numpy as np
import ml_dtypes

import concourse.bacc as bacc
import concourse.tile as tile
from concourse import mybir
from concourse.library_config import mlp

# Problem constants (hardcoded per contest rules).
N, D, V, K = 16384, 128, 1000000, 20
NCORES = 8
P = 128                      # SBUF partitions
KP = K + 1                   # pairs per sample (1 pos + 20 neg)
VS = V // NCORES             # table rows per core (125000)
CHUNK_ROWS = 16384           # table rows per gather call (int16 index range)
NCH = -(-VS // CHUNK_ROWS)   # 8 chunks per core

BF16 = mybir.dt.bfloat16
F32 = mybir.dt.float32
I16 = mybir.dt.int16

NUM_QUEUES = 4


def build_bass(l_fix, vs=VS, chunk_rows=CHUNK_ROWS, d=D):
    """Build the single-core SPMD Bass program with per-chunk capacity l_fix."""
    nch = -(-vs // chunk_rows)
    ntot = nch * l_fix
    nb = ntot // P               # score blocks per partition
    cb = l_fix // P              # blocks per chunk
    nc = bacc.Bacc("TRN2", target_bir_lowering=False, num_swdge_queues=NUM_QUEUES)
    t_table = nc.dram_tensor("table", [vs, d], BF16, kind="ExternalInput")
    t_hslot = nc.dram_tensor("hslot", [P, nb * d], BF16, kind="ExternalInput")
    t_tidx = nc.dram_tensor("tidx", [P, ntot // 16], I16, kind="ExternalInput")
    t_sig = nc.dram_tensor("sig", [P, nb], F32, kind="ExternalInput")
    t_wl = nc.dram_tensor("wl", [P, nb], F32, kind="ExternalInput")
    t_out = nc.dram_tensor("out", [P, 1], F32, kind="ExternalOutput")

    with (
        tile.TileContext(nc) as tc,
        tc.tile_pool(name="cpool", bufs=1) as cpool,
        tc.tile_pool(name="gpool", bufs=3) as gpool,
        tc.tile_pool(name="wpool", bufs=2) as wpool,
    ):
        nc.gpsimd.load_library(mlp)
        tidx = cpool.tile([P, ntot // 16], I16)
        sig = cpool.tile([P, nb], F32)
        nc.scalar.dma_start(out=sig[:], in_=t_sig[:])
        wl = cpool.tile([P, nb], F32)
        nc.scalar.dma_start(out=wl[:], in_=t_wl[:])
        scores = cpool.tile([P, nb], F32)

        signed = cpool.tile([P, nb], F32)
        ifree = l_fix // 16      # idx columns per chunk
        half = l_fix // 2        # rows per subgather (multiple of 128)
        # chunk-0 index slice first so the first gather starts early; the
        # rest of the indices follow as one transfer.
        nc.sync.dma_start(out=tidx[:, :ifree], in_=t_tidx[:, :ifree])
        nc.sync.dma_start(out=tidx[:, ifree:], in_=t_tidx[:, ifree:])
        for c in range(nch):
            csize = min(chunk_rows, vs - c * chunk_rows)
            r = gpool.tile([P, cb, d], BF16, tag="r")
            for s in range(2):
                s0 = s * half
                nc.gpsimd.dma_gather(
                    r[:, s0 // P : (s0 + half) // P, :],
                    t_table[c * chunk_rows : c * chunk_rows + csize, :],
                    tidx[:, c * ifree + s0 // 16 : c * ifree + (s0 + half) // 16],
                    half,
                    half,
                    d,
                    queue_num=(2 * c + s) % NUM_QUEUES,
                    single_packet=False,
                )
            h2 = gpool.tile([P, cb, d], BF16, tag="h2")
            # alternate the two HWDGE rings so hidden streaming isn't
            # serialized on one ring behind the index loads
            heng = nc.sync if c % 2 == 0 else nc.scalar
            heng.dma_start(
                out=h2[:].rearrange("p a d -> p (a d)"),
                in_=t_hslot[:, c * cb * d : (c + 1) * cb * d],
            )
            m = wpool.tile([P, cb, d], BF16, tag="m")
            nc.vector.tensor_mul(
                out=m[:].rearrange("p a d -> p (a d)"),
                in0=r[:].rearrange("p a d -> p (a d)"),
                in1=h2[:].rearrange("p a d -> p (a d)"),
            )
            cs = scores[:, c * cb : (c + 1) * cb]
            nc.vector.tensor_reduce(
                out=cs,
                in_=m[:],
                axis=mybir.AxisListType.X,
                op=mybir.AluOpType.add,
            )
            # signed = sig * score per chunk (overlaps the next gather);
            # the softplus chain runs once over the full width below so the
            # ACT engine loads each function table exactly once.
            nc.vector.tensor_mul(
                out=signed[:, c * cb : (c + 1) * cb],
                in0=cs,
                in1=sig[:, c * cb : (c + 1) * cb],
            )

        # softplus(x) = relu(x) + log1p(exp(-|x|)), batched over [P, nb]
        absx = cpool.tile([P, nb], F32)
        nc.scalar.activation(
            out=absx[:], in_=signed[:], func=mybir.ActivationFunctionType.Abs
        )
        expx = cpool.tile([P, nb], F32)
        nc.scalar.activation(
            out=expx[:],
            in_=absx[:],
            func=mybir.ActivationFunctionType.Exp,
            scale=-1.0,
        )
        lnx = cpool.tile([P, nb], F32)
        nc.scalar.activation(
            out=lnx[:],
            in_=expx[:],
            func=mybir.ActivationFunctionType.Ln,
            bias=1.0,
        )
        sp = cpool.tile([P, nb], F32)
        nc.vector.scalar_tensor_tensor(
            out=sp[:],
            in0=signed[:],
            scalar=0.0,
            in1=lnx[:],
            op0=mybir.AluOpType.max,
            op1=mybir.AluOpType.add,
        )
        contrib = cpool.tile([P, nb], F32)
        nc.vector.tensor_mul(out=contrib[:], in0=sp[:], in1=wl[:])
        partial = cpool.tile([P, 1], F32)
        nc.vector.tensor_reduce(
            out=partial[:],
            in_=contrib[:],
            axis=mybir.AxisListType.X,
            op=mybir.AluOpType.add,
        )
        nc.sync.dma_start(out=t_out[:], in_=partial[:])

    nc.compile()
    return nc


def _wrap_idx16(flat):
    """flat[n] -> idx tile [128, len//16]: value n at (partition n%16, col n//16),
    replicated across the 8 groups of 16 partitions."""
    m = flat.reshape(-1, 16).T.astype(np.int16)
    return np.ascontiguousarray(np.tile(m, (8, 1)))


def _block_layout(flat):
    """flat[n] -> [128, len//128] with value n at (partition n%128, col n//128)."""
    return np.ascontiguousarray(flat.reshape(-1, P).T)


def _split_pairs(label_idxes, neg_idxes):
    """Global pair arrays: table idx, sample idx, sigma, weight."""
    pairs = np.concatenate(
        [np.asarray(label_idxes, np.int32)[:, None], np.asarray(neg_idxes, np.int32)],
        axis=1,
    )  # [N, KP]
    tidx = pairs.reshape(-1)
    samp = np.repeat(np.arange(N, dtype=np.int32), KP)
    sig = np.tile(np.array([-1.0] + [1.0] * K, np.float32), N)
    wl = np.tile(np.array([1.0] + [1.0 / K] * K, np.float32), N)
    return tidx, samp, sig, wl


def prep_core_inputs(tidx, samp, sig, wl, l_fix, hidden_bf16,
                     vs=VS, chunk_rows=CHUNK_ROWS):
    """Sort one core's pairs (local table idx) by chunk, pad each to l_fix."""
    nch = -(-vs // chunk_rows)
    ntot = nch * l_fix
    order = np.argsort(tidx, kind="stable")
    s_tidx = tidx[order]
    s_samp = samp[order]
    s_sig = sig[order]
    s_wl = wl[order]
    chunk = s_tidx // chunk_rows
    counts = np.bincount(chunk, minlength=nch)
    if counts.max() > l_fix:
        raise OverflowError(int(counts.max()))
    g_tidx = np.zeros(ntot, np.int32)
    g_samp = np.zeros(ntot, np.int32)
    g_sig = np.ones(ntot, np.float32)
    g_wl = np.zeros(ntot, np.float32)
    starts = np.concatenate([[0], np.cumsum(counts)])
    for c in range(nch):
        a, b = starts[c], starts[c + 1]
        o = c * l_fix
        g_tidx[o : o + b - a] = s_tidx[a:b] - c * chunk_rows
        g_samp[o : o + b - a] = s_samp[a:b]
        g_sig[o : o + b - a] = s_sig[a:b]
        g_wl[o : o + b - a] = s_wl[a:b]
    # hidden rows replicated into slot order: [P, nb, d] with slot
    # (block b, partition p) = global slot b*128 + p, matching the
    # dma_gather output layout.
    hs = hidden_bf16[g_samp]                       # [ntot, d]
    hs = hs.reshape(-1, P, D).transpose(1, 0, 2)   # [P, nb, d]
    return {
        "tidx": _wrap_idx16(g_tidx),
        "hslot": np.ascontiguousarray(hs.reshape(P, -1)),
        "sig": _block_layout(g_sig),
        "wl": _block_layout(g_wl),
    }


def plan_capacity(tidx):
    """Max pairs in any (core, chunk), rounded up to a multiple of 256
    (so the per-chunk capacity splits into two 128-aligned subgathers)."""
    core = tidx // VS
    local = tidx - core * VS
    chunk = local // CHUNK_ROWS
    counts = np.bincount(core * NCH + chunk, minlength=NCORES * NCH)
    return int(-(-counts.max() // (2 * P)) * (2 * P))


def make_in_maps(hidden_state, label_idxes, neg_idxes, out_embed_weight, l_fix):
    table_bf16 = np.ascontiguousarray(out_embed_weight).astype(ml_dtypes.bfloat16)
    hidden_bf16 = np.ascontiguousarray(hidden_state).astype(ml_dtypes.bfloat16)
    tidx, samp, sig, wl = _split_pairs(label_idxes, neg_idxes)
    core_of_pair = tidx // VS
    in_maps = []
    for c in range(NCORES):
        sel = core_of_pair == c
        core = prep_core_inputs(
            tidx[sel] - c * VS, samp[sel], sig[sel], wl[sel], l_fix, hidden_bf16
        )
        core["table"] = np.ascontiguousarray(table_bf16[c * VS : (c + 1) * VS])
        in_maps.append(core)
    return in_maps


_NC_CACHE = {}


def get_nc(l_fix):
    key = l_fix
    if key not in _NC_CACHE:
        _NC_CACHE[key] = build_bass(l_fix)
    return _NC_CACHE[key]


def kernel(hidden_state, label_idxes, neg_idxes, out_embed_weight):
    from concourse.bass_utils import run_bass_kernel_spmd

    tidx_flat, _, _, _ = _split_pairs(label_idxes, neg_idxes)
    l_fix = plan_capacity(tidx_flat)
    nc = get_nc(l_fix)
    in_maps = make_in_maps(
        hidden_state, label_idxes, neg_idxes, out_embed_weight, l_fix
    )
    res = run_bass_kernel_spmd(nc, in_maps, core_ids=list(range(NCORES)))
    total = 0.0
    for r in res.results:
        total += float(np.asarray(r["out"], np.float64).sum())
    return np.float32(total)
